# revision 16
# baseline (speedup 1.0000x reference)
"""Bass/Trainium2 kernel for DirSAGEEmbRes (2-layer directed SAGE + residual).

Strategy (8 NeuronCores, SPMD):
  - dst nodes sharded 1/8 per core; edge lists bucketed per (core, dst-window,
    src-quarter) on host, padded so all cores share one compile-time layout.
  - features are pre-transformed before aggregation (z = x0 @ Wl), so every
    gathered row is exactly 128 bf16 = 256B (the dma_gather quantum), and
    mean(z[src]) == mean(x0[src]) @ Wl by linearity.
  - segment-mean: dma_gather (bf16 rows) + PE one-hot scatter matmuls into
    PSUM; one-hot matrices built in a single batched 4x-mode DVE op per
    segment; 1/deg folded in on eviction via ACT scale; evicted means are
    transposed straight into the dense-layer PSUM accumulation.
  - z / e1 tables exchanged via async AllGather, overlapped with the next
    phase's compute.
"""
import os
import sys

sys.path.insert(0, "/opt/trn_rl_repo")

import numpy as np
import ml_dtypes

import concourse.bass as bass
import concourse.bacc as bacc
import concourse.mybir as mybir
from concourse.tile import TileContext
from concourse.library_config import mlp
from concourse.masks import make_identity

BF = ml_dtypes.bfloat16
NCORE = 8
GW = 8          # dst windows per reduce group (2 PSUM bank-tiles of 4)
HID = 128
D0 = 144


# ----------------------------------------------------------------- host prep

def _ceil(a, b):
    return -(-a // b)


class PassLayout:
    """Compile-time layout of one aggregation pass (shared by all cores)."""

    def __init__(self, L_wq, W):
        # L_wq: [W, 4] per-(window, quarter) run lengths (max over cores)
        self.W = W
        self.NG = _ceil(W, GW)
        self.L_wq = L_wq
        self.segs = []          # (g, q, idx_pos, seglen, dst_col0, mms)
        self.run_start = np.zeros((W, 4), np.int64)
        self.seg_start = {}      # (g, q) -> (idx stream position, dst col0)
        idx_pos = 0
        dst_col = 0
        for g in range(self.NG):
            wins = list(range(g * GW, min((g + 1) * GW, W)))
            for q in range(4):
                runs = [int(L_wq[w, q]) for w in wins]
                total = int(np.sum(runs))
                seglen = _ceil(max(total, 1), 128) * 128
                off = 0
                spans = []
                for w, r in zip(wins, runs):
                    self.run_start[w, q] = off
                    spans.append((w, off, off + r))
                    off += r
                # matmul list: (block, window, local col)
                mms = []
                col = 0
                for b in range(seglen // 128):
                    lo, hi = b * 128, b * 128 + 128
                    for (w, s, e) in spans:
                        if s < hi and e > lo:
                            mms.append((b, w, col))
                            col += 1
                self.seg_start[(g, q)] = (idx_pos, dst_col)
                self.segs.append((g, q, idx_pos, seglen, dst_col, mms))
                idx_pos += seglen
                dst_col += col
        self.tot_idx = idx_pos
        self.tot_col = dst_col
        self.max_ncols = max((len(s[5]) for s in self.segs), default=1)


def _build_pass(src_q, src_iq, core, w, woff, layout):
    """Build per-core idx (int16, wrap layout) + dstoff (bf16) streams."""
    W = layout.W
    idx_streams = np.zeros((NCORE, layout.tot_idx), np.int16)
    dst_streams = np.full((NCORE, layout.tot_col, 128), -1.0, np.float32)
    order = np.lexsort((src_iq, src_q, w, core))
    c_s, w_s, q_s, iq_s, woff_s = (
        core[order], w[order], src_q[order], src_iq[order], woff[order])
    key = ((c_s * W + w_s) * 4 + q_s).astype(np.int64)
    uniq, first = np.unique(key, return_index=True)
    rank = np.arange(len(key)) - first[np.searchsorted(uniq, key)]
    g_s = w_s // GW
    seg0 = np.zeros(len(key), np.int64)
    for (g, q), (ip, dc) in layout.seg_start.items():
        m = (g_s == g) & (q_s == q)
        seg0[m] = ip
    pos_in_seg = layout.run_start[w_s, q_s] + rank
    pos = seg0 + pos_in_seg
    for c in range(NCORE):
        m = c_s == c
        idx_streams[c, pos[m]] = iq_s[m].astype(np.int16)
    bl = pos_in_seg // 128
    eoff = pos_in_seg % 128
    NG = layout.NG
    maxb = max((s[3] // 128 for s in layout.segs), default=1)
    colarr = np.full((NG, 4, maxb, W), -1, np.int64)
    for (g, q, ip, seglen, dc, mms) in layout.segs:
        for (b, wv, col) in mms:
            colarr[g, q, b, wv] = dc + col
    cols = colarr[g_s, q_s, bl, w_s]
    assert (cols >= 0).all()
    for c in range(NCORE):
        m = c_s == c
        dst_streams[c, cols[m], eoff[m]] = woff_s[m].astype(np.float32)
    # wrap idx: unwrapped[i] = wrap[i%16, i//16], tiled to 128 partitions
    idx_wrap = np.ascontiguousarray(
        np.tile(idx_streams.reshape(NCORE, layout.tot_idx // 16, 16)
                .transpose(0, 2, 1), (1, 8, 1)))
    dst_t = np.ascontiguousarray(
        dst_streams.transpose(0, 2, 1))  # [8,128,cols] f32
    return idx_wrap, dst_t


def _preprocess(edge, N, NPC, NPCP, W, QSH):
    """Per direction: one layout (used by both layers) + streams + inv."""
    src = edge[0].astype(np.int64)
    dst = edge[1].astype(np.int64)
    core = dst // NPC
    ldst = dst - core * NPC
    w = ldst // 128
    woff = ldst % 128

    cnt = np.bincount(core * NPCP + ldst, minlength=NCORE * NPCP).reshape(NCORE, NPCP)
    inv = np.where(cnt > 0, 1.0 / np.maximum(cnt, 1), 0.0).astype(np.float32)
    inv_t = np.ascontiguousarray(
        inv.reshape(NCORE, W, 128).transpose(0, 2, 1))  # [8, 128, W]

    # src indices into the AllGathered table [(src//NPC)*NPCP + src%NPC]
    pos = (src // NPC) * NPCP + (src % NPC)
    q = pos // QSH
    iq = pos - q * QSH
    counts = np.bincount(((core * W + w) * 4 + q).astype(np.int64),
                         minlength=NCORE * W * 4).reshape(NCORE, W, 4)
    layout = PassLayout(counts.max(axis=0), W)
    idx_wrap, dst_t = _build_pass(q, iq, core, w, woff, layout)
    return layout, idx_wrap, dst_t, inv_t


# -------------------------------------------------------------- bass builder

def _build_nc(NPC, NPCP, W, QSH, NTH, NBMAX, layouts):
    """layouts: dict[d] -> PassLayout"""
    f32 = mybir.dt.float32
    bf16 = mybir.dt.bfloat16
    AF = mybir.ActivationFunctionType
    ALU = mybir.AluOpType

    nc = bacc.Bacc(None, target_bir_lowering=False, debug=False,
                   num_swdge_queues=2)
    t_x0T = nc.dram_tensor("x0T", [128, 2 * NPCP], bf16, kind="ExternalInput")
    t_iota2 = nc.dram_tensor("iota2", [128, 128], bf16, kind="ExternalInput")
    t_idx, t_dst, t_inv = {}, {}, {}
    for d in range(2):
        lo = layouts[d]
        t_idx[d] = nc.dram_tensor(f"idx_{d}", [128, lo.tot_idx // 16],
                                  mybir.dt.int16, kind="ExternalInput")
        t_dst[d] = nc.dram_tensor(f"dst_{d}", [128, lo.tot_col], f32,
                                  kind="ExternalInput")
        t_inv[d] = nc.dram_tensor(f"inv_{d}", [128, W], f32,
                                  kind="ExternalInput")
    names = ["in1", "in2", "out1", "out2"]
    t_w = {}
    for nm in names:
        d_in = D0 if nm.endswith("1") else HID
        t_w[nm + "_Wl"] = nc.dram_tensor(nm + "_Wl", [d_in, HID], f32, kind="ExternalInput")
        t_w[nm + "_Wr"] = nc.dram_tensor(nm + "_Wr", [d_in, HID], f32, kind="ExternalInput")
        t_w[nm + "_bl"] = nc.dram_tensor(nm + "_bl", [HID], f32, kind="ExternalInput")
    t_lin = nc.dram_tensor("lin_W", [2 * HID], f32, kind="ExternalInput")
    t_linb = nc.dram_tensor("lin_b", [128], f32, kind="ExternalInput")
    t_y = nc.dram_tensor("y", [NPCP], f32, kind="ExternalOutput")

    with TileContext(nc) as tc:
        with (
            tc.tile_pool(name="const", bufs=1) as constp,
            tc.tile_pool(name="stage", bufs=2) as stagep,
            tc.tile_pool(name="sS", bufs=6) as sp,
            tc.tile_pool(name="small", bufs=3) as smallp,
            tc.tile_pool(name="mns", bufs=6) as mnsp,
            tc.tile_pool(name="big", bufs=1) as bigp,
            tc.tile_pool(name="ps", bufs=6, space="PSUM") as psp,
            tc.tile_pool(name="pst", bufs=2, space="PSUM") as pstp,
            tc.tile_pool(name="dram", bufs=1, space="DRAM") as dramp,
        ):
            nc.gpsimd.load_library(mlp)
            # constants
            ident = constp.tile([128, 128], f32, tag="ident")
            make_identity(nc, ident[:])
            ident_bf = constp.tile([128, 128], bf16, tag="identbf")
            nc.vector.tensor_copy(out=ident_bf[:], in_=ident[:])
            iota = constp.tile([128, 128], bf16, tag="iota2")
            nc.sync.dma_start(out=iota[:], in_=t_iota2[:])
            invt = {}
            for d in range(2):
                invt[d] = constp.tile([128, W], f32, tag=f"inv{d}", name=f"inv{d}")
                nc.sync.dma_start(out=invt[d][:], in_=t_inv[d][:])
            # weights: load f32 then convert to bf16 once
            wt = {}
            for nm in names:
                for side in ("Wl", "Wr"):
                    src_t = t_w[nm + "_" + side]
                    a32 = smallp.tile([128, 128], f32, tag="w32")
                    nc.sync.dma_start(out=a32[:], in_=src_t[0:128, :])
                    a = constp.tile([128, 128], bf16, tag=f"{nm}{side}a",
                                    name=f"{nm}{side}a")
                    nc.vector.tensor_copy(out=a[:], in_=a32[:])
                    wt[nm + side + "a"] = a
                    if nm.endswith("1"):
                        b32 = smallp.tile([128, 128], f32, tag="w32")
                        nc.sync.dma_start(out=b32[0:16, :], in_=src_t[128:144, :])
                        b = constp.tile([128, 128], bf16, tag=f"{nm}{side}b",
                                        name=f"{nm}{side}b")
                        nc.vector.tensor_copy(out=b[0:16, :], in_=b32[0:16, :])
                        wt[nm + side + "b"] = b
                bt = constp.tile([128, 1], f32, tag=f"{nm}bl", name=f"{nm}bl")
                nc.sync.dma_start(out=bt[:], in_=t_w[nm + "_bl"][:, None])
                wt[nm + "bl"] = bt
            lin_f = constp.tile([128, 2], f32, tag="linf")
            nc.sync.dma_start(out=lin_f[:], in_=t_lin.rearrange("(h p) -> p h", p=128))
            lin_bf = constp.tile([128, 2], bf16, tag="linbf")
            nc.vector.tensor_copy(out=lin_bf[:], in_=lin_f[:])
            linb_sb = constp.tile([128, 1], f32, tag="linb")
            nc.sync.dma_start(out=linb_sb[:], in_=t_linb[:, None])

            y_sb = constp.tile([128, W], f32, tag="ysb")

            h1T = [bigp.tile([128, NPCP], bf16, tag=f"h1T{d}", name=f"h1T{d}")
                   for d in range(2)]
            z_own = [dramp.tile([NPCP, HID], bf16, tag=f"zown{d}", name=f"zown{d}")
                     for d in range(2)]
            e_own = [dramp.tile([NPCP, HID], bf16, tag=f"eown{d}", name=f"eown{d}")
                     for d in range(2)]
            z_ag = [dramp.tile([NTH, HID], bf16, tag=f"zag{d}", name=f"zag{d}",
                               addr_space="Shared") for d in range(2)]
            e_ag = [dramp.tile([NTH, HID], bf16, tag=f"eag{d}", name=f"eag{d}",
                               addr_space="Shared") for d in range(2)]

            qctr = [0]
            NTILE = _ceil(NPCP, 512)

            def export_tile(srcT, src_off, nn, dram_own, node0):
                """srcT[:, src_off:+nn] (bf16, feature-major) -> node-major DRAM
                rows node0..node0+nn."""
                nwin = nn // 128
                hn = smallp.tile([128, 512], bf16, tag="hn")
                for k in range(nwin):
                    tp = pstp.tile([128, 256], bf16, tag="tr", name="tpbf")
                    nc.tensor.transpose(
                        tp[:, 0:128],
                        srcT[:, src_off + k * 128:src_off + (k + 1) * 128],
                        ident_bf[:])
                    nc.vector.tensor_copy(out=hn[:, k * 128:(k + 1) * 128],
                                          in_=tp[:, 0:128])
                w0 = node0 // 128
                nc.sync.dma_start(
                    out=dram_own[:].rearrange("(w p) j -> p w j", p=128)[:, w0:w0 + nwin, :],
                    in_=hn[:, 0:nn].rearrange("p (w j) -> p w j", j=128))

            def transform_pass(d, lhsT_list, src_rhs, dram_own):
                """out = sum_i lhsT_i^T @ rhs_i per 512-tile, export node-major."""
                for ti in range(NTILE):
                    n0 = ti * 512
                    nn = min(512, NPCP - n0)
                    ps = psp.tile([128, 512], f32, tag="red")
                    rhss = src_rhs(n0, nn)
                    for i, (lhsT, rhs) in enumerate(zip(lhsT_list, rhss)):
                        nc.tensor.matmul(ps[:, 0:nn], lhsT=lhsT, rhs=rhs,
                                         start=(i == 0),
                                         stop=(i == len(lhsT_list) - 1))
                    zt = smallp.tile([128, 512], bf16, tag="zt")
                    nc.scalar.activation(out=zt[:, 0:nn], in_=ps[:, 0:nn],
                                         func=AF.Copy)
                    export_tile(zt, 0, nn, dram_own, n0)

            def load_xa(n0, nn):
                xa = smallp.tile([128, 512], bf16, tag="xa")
                nc.sync.dma_start(out=xa[:, 0:nn], in_=t_x0T[:, n0:n0 + nn])
                xb = smallp.tile([128, 512], bf16, tag="xb")
                nc.sync.dma_start(out=xb[0:16, 0:nn],
                                  in_=t_x0T[0:16, NPCP + n0:NPCP + n0 + nn])
                return xa, xb

            def reduce_dense(d, table_ap, dense_mms, bias, out_cb):
                """Aggregation pass + fused dense layer.

                dense_mms(n0, nn) -> list of (lhsT, rhs) accumulated with the
                transposed means into one PSUM tile; out_cb(ps, n0, nn)
                consumes the finished PSUM (adding bias/relu)."""
                lo = layouts[d]
                for g in range(lo.NG):
                    wins = list(range(g * GW, min((g + 1) * GW, lo.W)))
                    nk = _ceil(len(wins), 4)
                    ptiles = [psp.tile([128, 512], f32, tag="red",
                                       name=f"agg{g}_{k}") for k in range(nk)]
                    for t in ptiles:
                        nc.vector.memset(t[:], 0.0)
                    segs = [s for s in lo.segs if s[0] == g]
                    # per-bank first/last matmul bookkeeping (PSUM groups are
                    # bank-granular: interleaved per-region groups lose
                    # contributions — probed on HW)
                    wcount = {wv: 0 for wv in wins}
                    kcount = [0] * nk
                    for (_, _, _, _, _, mms) in segs:
                        for (_, wv, _) in mms:
                            wcount[wv] += 1
                            kcount[wins.index(wv) // 4] += 1
                    kseen = [0] * nk
                    for (gg, q, ip, seglen, dc, mms) in segs:
                        nb = seglen // 128
                        ncols = len(mms)
                        idxs = smallp.tile([128, seglen // 16], mybir.dt.int16,
                                           tag="idxs")
                        nc.sync.dma_start(
                            out=idxs[:],
                            in_=t_idx[d][:, ip // 16: ip // 16 + seglen // 16])
                        dstc = smallp.tile([128, max(ncols, 1)], f32, tag="dstc")
                        if ncols:
                            nc.sync.dma_start(
                                out=dstc[:, 0:ncols],
                                in_=t_dst[d][:, dc:dc + ncols])
                        stage = stagep.tile([128, nb * 128], bf16, tag="stage")
                        nc.gpsimd.dma_gather(
                            stage[:].rearrange("p (b e) -> p b e", e=128),
                            table_ap[q * QSH:(q + 1) * QSH],
                            idxs[:],
                            seglen, seglen, 128,
                            single_packet=False,
                            queue_num=qctr[0] % 2,
                        )
                        qctr[0] += 1
                        for (b, wv, col) in mms:
                            wl = wins.index(wv)
                            k = wl // 4
                            kseen[k] += 1
                            S = sp.tile([128, 128], bf16, tag="S")
                            nc.vector.tensor_scalar(
                                out=S[:], in0=iota[:],
                                scalar1=dstc[:, col:col + 1], scalar2=None,
                                op0=ALU.is_equal)
                            nc.tensor.matmul(
                                ptiles[k][:, (wl % 4) * 128:(wl % 4) * 128 + 128],
                                lhsT=S[:],
                                rhs=stage[:, b * 128:b * 128 + 128],
                                start=(kseen[k] == 1),
                                stop=(kseen[k] == kcount[k]))
                    # evictions + fused dense per 4-window node tile
                    for k in range(nk):
                        wk = wins[4 * k:4 * k + 4]
                        n0 = wk[0] * 128
                        nn = len(wk) * 128
                        mts = []
                        for j, wv in enumerate(wk):
                            wl = 4 * k + j
                            mn = mnsp.tile([128, 128], f32, tag="mn")
                            nc.scalar.activation(
                                out=mn[:],
                                in_=ptiles[k][:, (wl % 4) * 128:(wl % 4) * 128 + 128],
                                func=AF.Copy,
                                scale=invt[d][:, wv:wv + 1])
                            mts.append(mn)
                        ps = psp.tile([128, 512], f32, tag="red",
                                      name=f"dense{g}_{k}")
                        dm = dense_mms(n0, nn)
                        for i, (lhsT, rhs) in enumerate(dm):
                            nc.tensor.matmul(ps[:, 0:nn], lhsT=lhsT, rhs=rhs,
                                             start=(i == 0), stop=False)
                        for j, mn in enumerate(mts):
                            nc.tensor.matmul(
                                ps[:, j * 128:(j + 1) * 128],
                                lhsT=mn[:], rhs=ident[:],
                                is_transpose=True,
                                start=False, stop=(j == len(mts) - 1))
                        out_cb(ps, n0, nn)

            for d in range(2):
                nm1, nm2 = names[2 * d], names[2 * d + 1]
                # ---- z-pass: z = x0 @ Wl1, export + AllGather ----
                transform_pass(
                    d,
                    [wt[nm1 + "Wla"][:], wt[nm1 + "Wlb"][0:16, :]],
                    lambda n0, nn: [x[0:128, 0:nn] if i == 0 else x[0:16, 0:nn]
                                    for i, x in enumerate(load_xa(n0, nn))],
                    z_own[d])
                nc.gpsimd.collective_compute(
                    "AllGather", mybir.AluOpType.bypass,
                    replica_groups=[list(range(NCORE))],
                    ins=[z_own[d][:]], outs=[z_ag[d][:]])

            for d in range(2):
                nm1, nm2 = names[2 * d], names[2 * d + 1]

                # ---- layer 1: mean(z) + x0 @ Wr1, relu -> h1T ----
                def l1_dense(n0, nn):
                    xa, xb = load_xa(n0, nn)
                    return [(wt[nm1 + "Wra"][:], xa[:, 0:nn]),
                            (wt[nm1 + "Wrb"][0:16, :], xb[0:16, 0:nn])]

                def l1_out(ps, n0, nn, d=d, nm1=nm1):
                    nc.scalar.activation(out=h1T[d][:, n0:n0 + nn],
                                         in_=ps[:, 0:nn],
                                         func=AF.Relu,
                                         bias=wt[nm1 + "bl"][:, 0:1], scale=1.0)

                reduce_dense(d, z_ag[d][:], l1_dense, wt[nm1 + "bl"], l1_out)

                # ---- e1-pass: e1 = h1 @ Wl2, export + AllGather ----
                transform_pass(
                    d,
                    [wt[nm2 + "Wla"][:]],
                    lambda n0, nn: [h1T[d][:, n0:n0 + nn]],
                    e_own[d])
                nc.gpsimd.collective_compute(
                    "AllGather", mybir.AluOpType.bypass,
                    replica_groups=[list(range(NCORE))],
                    ins=[e_own[d][:]], outs=[e_ag[d][:]])

            for d in range(2):
                nm1, nm2 = names[2 * d], names[2 * d + 1]

                # ---- layer 2: mean(e1) + h1 @ Wr2, relu, residual ----
                def l2_dense(n0, nn, d=d, nm2=nm2):
                    return [(wt[nm2 + "Wra"][:], h1T[d][:, n0:n0 + nn])]

                def l2_out(ps, n0, nn, d=d, nm2=nm2):
                    h2t = smallp.tile([128, 512], bf16, tag="h2t")
                    nc.scalar.activation(out=h2t[:, 0:nn], in_=ps[:, 0:nn],
                                         func=AF.Relu,
                                         bias=wt[nm2 + "bl"][:, 0:1], scale=1.0)
                    nc.vector.tensor_add(out=h1T[d][:, n0:n0 + nn],
                                         in0=h1T[d][:, n0:n0 + nn],
                                         in1=h2t[:, 0:nn])

                reduce_dense(d, e_ag[d][:], l2_dense, wt[nm2 + "bl"], l2_out)

            # ---------------- y = h_in @ lin[:128] + h_out @ lin[128:] + b ----
            for wv in range(W):
                yp = psp.tile([128, 512], f32, tag="red", name="yp")
                nc.tensor.matmul(yp[:, 0:1],
                                 lhsT=h1T[0][:, wv * 128:(wv + 1) * 128],
                                 rhs=lin_bf[:, 0:1], start=True, stop=False)
                nc.tensor.matmul(yp[:, 0:1],
                                 lhsT=h1T[1][:, wv * 128:(wv + 1) * 128],
                                 rhs=lin_bf[:, 1:2], start=False, stop=True)
                nc.scalar.activation(out=y_sb[:, wv:wv + 1], in_=yp[:, 0:1],
                                     func=AF.Copy)
            nc.vector.tensor_scalar(
                out=y_sb[:], in0=y_sb[:],
                scalar1=linb_sb[:, 0:1], scalar2=None,
                op0=ALU.add)
            nc.sync.dma_start(out=t_y.rearrange("(w p) -> p w", p=128), in_=y_sb[:])

    nc.compile()
    return nc


# ------------------------------------------------------------------ wrapper

def _prep_all(x, edge_in, edge_out, emb):
    N = x.shape[0]
    NPC = N // NCORE
    W = _ceil(NPC, 128)
    NPCP = W * 128
    QSH = 2 * NPCP
    NTH = NCORE * NPCP

    x = np.asarray(x, np.float32)
    emb = np.asarray(emb, np.float32)

    pre = {}
    for d, edge in enumerate((edge_in, edge_out)):
        pre[d] = _preprocess(np.asarray(edge), N, NPC, NPCP, W, QSH)

    NBMAX = max(max(pre[d][0].max_ncols for d in range(2)), 2)
    iota2 = np.broadcast_to(np.arange(128, dtype=np.float32),
                            (128, 128)).astype(BF)

    x0T = np.zeros((NCORE, 128, 2 * NPCP), np.float32)
    for c in range(NCORE):
        blk = np.zeros((NPCP, D0), np.float32)
        blk[:NPC, 0:128] = x[c * NPC:(c + 1) * NPC]
        blk[:NPC, 128:144] = emb[c * NPC:(c + 1) * NPC]
        x0T[c, :, :NPCP] = blk[:, 0:128].T
        x0T[c, 0:16, NPCP:] = blk[:, 128:144].T
    x0T = x0T.astype(BF)

    layouts = {d: pre[d][0] for d in range(2)}
    dims = dict(N=N, NPC=NPC, NPCP=NPCP, W=W, QSH=QSH, NTH=NTH, NBMAX=NBMAX)
    return dims, layouts, pre, x0T, iota2


def _in_maps(dims, pre, x0T, iota2, kw):
    maps = []
    for c in range(NCORE):
        m = {"x0T": np.ascontiguousarray(x0T[c]), "iota2": np.asarray(iota2)}
        for d in range(2):
            _, idx_wrap, dst_t, inv_t = pre[d]
            m[f"idx_{d}"] = np.ascontiguousarray(idx_wrap[c])
            m[f"dst_{d}"] = np.ascontiguousarray(dst_t[c])
            m[f"inv_{d}"] = np.ascontiguousarray(inv_t[c])
        for nm in ("in1", "in2", "out1", "out2"):
            m[nm + "_Wl"] = np.asarray(kw[nm + "_Wl"], np.float32)
            m[nm + "_Wr"] = np.asarray(kw[nm + "_Wr"], np.float32)
            m[nm + "_bl"] = np.asarray(kw[nm + "_bl"], np.float32)
        m["lin_W"] = np.asarray(kw["lin_W"], np.float32).reshape(-1)
        m["lin_b"] = np.full(128, np.asarray(kw["lin_b"], np.float32).reshape(-1)[0], np.float32)
        maps.append(m)
    return maps


def kernel(x, edge_in, edge_out, emb, **kw):
    from concourse.bass_utils import run_bass_kernel_spmd
    dims, layouts, pre, x0T, iota2 = _prep_all(x, edge_in, edge_out, emb)
    nc = _build_nc(dims["NPC"], dims["NPCP"], dims["W"], dims["QSH"],
                   dims["NTH"], dims["NBMAX"], layouts)
    maps = _in_maps(dims, pre, x0T, iota2, kw)
    res = run_bass_kernel_spmd(nc, maps, core_ids=list(range(NCORE)))
    NPC = dims["NPC"]
    y = np.empty(dims["N"], np.float32)
    for c in range(NCORE):
        y[c * NPC:(c + 1) * NPC] = res.results[c]["y"][:NPC]
    return y


# revision 22
# speedup vs baseline: 2.0674x; 2.0674x over previous
"""Bass/Trainium2 kernel for DirSAGEEmbRes (2-layer directed SAGE + residual).

Strategy (8 NeuronCores, SPMD):
  - dst nodes sharded 1/8 per core; edge lists bucketed per (core, dst-window,
    src-quarter) on host, padded so all cores share one compile-time layout.
  - features are pre-transformed before aggregation (z = x0 @ Wl), so every
    gathered row is exactly 128 bf16 = 256B (the dma_gather quantum), and
    mean(z[src]) == mean(x0[src]) @ Wl by linearity.
  - segment-mean: dma_gather (bf16 rows) + PE one-hot scatter matmuls into
    PSUM; one-hot matrices built in a single batched 4x-mode DVE op per
    segment; 1/deg folded in on eviction via ACT scale; evicted means are
    transposed straight into the dense-layer PSUM accumulation.
  - z / e1 tables exchanged via async AllGather, overlapped with the next
    phase's compute.
"""
import os
import sys

sys.path.insert(0, "/opt/trn_rl_repo")

import numpy as np
import ml_dtypes

import concourse.bass as bass
import concourse.bacc as bacc
import concourse.mybir as mybir
from concourse.tile import TileContext
from concourse.library_config import mlp
from concourse.masks import make_identity

BF = ml_dtypes.bfloat16
NCORE = 8
GW = 16         # dst windows per reduce group (4 PSUM bank-tiles of 4)
HID = 128
D0 = 144
# diagnostics: 1 = skip one-hot builds (S stays stale), 2 = also skip scatter
# matmuls; output is wrong — timing only
DIAG = int(os.environ.get("KDIAG", "0"))


# ----------------------------------------------------------------- host prep

def _ceil(a, b):
    return -(-a // b)


class PassLayout:
    """Compile-time layout of one aggregation pass (shared by all cores)."""

    def __init__(self, L_wq, W):
        # L_wq: [W, 4] per-(window, quarter) run lengths (max over cores)
        self.W = W
        self.NG = _ceil(W, GW)
        self.L_wq = L_wq
        self.segs = []          # (g, q, idx_pos, seglen, dst_col0, mms)
        self.run_start = np.zeros((W, 4), np.int64)
        self.seg_start = {}      # (g, q) -> (idx stream position, dst col0)
        idx_pos = 0
        dst_col = 0
        for g in range(self.NG):
            wins = list(range(g * GW, min((g + 1) * GW, W)))
            for q in range(4):
                runs = [int(L_wq[w, q]) for w in wins]
                total = int(np.sum(runs))
                seglen = _ceil(max(total, 1), 128) * 128
                off = 0
                spans = []
                for w, r in zip(wins, runs):
                    self.run_start[w, q] = off
                    spans.append((w, off, off + r))
                    off += r
                # matmul list: (block, window, local col)
                mms = []
                col = 0
                for b in range(seglen // 128):
                    lo, hi = b * 128, b * 128 + 128
                    for (w, s, e) in spans:
                        if s < hi and e > lo:
                            mms.append((b, w, col))
                            col += 1
                self.seg_start[(g, q)] = (idx_pos, dst_col)
                self.segs.append((g, q, idx_pos, seglen, dst_col, mms))
                idx_pos += seglen
                dst_col += col
        self.tot_idx = idx_pos
        self.tot_col = dst_col
        self.max_ncols = max((len(s[5]) for s in self.segs), default=1)


def _build_pass(src_q, src_iq, core, w, woff, layout):
    """Build per-core idx (int16, wrap layout) + dstoff (bf16) streams."""
    W = layout.W
    idx_streams = np.zeros((NCORE, layout.tot_idx), np.int16)
    dst_streams = np.full((NCORE, layout.tot_col, 128), -1.0, np.float32)
    order = np.lexsort((src_iq, src_q, w, core))
    c_s, w_s, q_s, iq_s, woff_s = (
        core[order], w[order], src_q[order], src_iq[order], woff[order])
    key = ((c_s * W + w_s) * 4 + q_s).astype(np.int64)
    uniq, first = np.unique(key, return_index=True)
    rank = np.arange(len(key)) - first[np.searchsorted(uniq, key)]
    g_s = w_s // GW
    seg0 = np.zeros(len(key), np.int64)
    for (g, q), (ip, dc) in layout.seg_start.items():
        m = (g_s == g) & (q_s == q)
        seg0[m] = ip
    pos_in_seg = layout.run_start[w_s, q_s] + rank
    pos = seg0 + pos_in_seg
    for c in range(NCORE):
        m = c_s == c
        idx_streams[c, pos[m]] = iq_s[m].astype(np.int16)
    bl = pos_in_seg // 128
    eoff = pos_in_seg % 128
    NG = layout.NG
    maxb = max((s[3] // 128 for s in layout.segs), default=1)
    colarr = np.full((NG, 4, maxb, W), -1, np.int64)
    for (g, q, ip, seglen, dc, mms) in layout.segs:
        for (b, wv, col) in mms:
            colarr[g, q, b, wv] = dc + col
    cols = colarr[g_s, q_s, bl, w_s]
    assert (cols >= 0).all()
    for c in range(NCORE):
        m = c_s == c
        dst_streams[c, cols[m], eoff[m]] = woff_s[m].astype(np.float32)
    # wrap idx: unwrapped[i] = wrap[i%16, i//16], tiled to 128 partitions
    idx_wrap = np.ascontiguousarray(
        np.tile(idx_streams.reshape(NCORE, layout.tot_idx // 16, 16)
                .transpose(0, 2, 1), (1, 8, 1)))
    dst_t = np.ascontiguousarray(
        dst_streams.transpose(0, 2, 1))  # [8,128,cols] f32
    return idx_wrap, dst_t


def _preprocess(edge, N, NPC, NPCP, W, QSH):
    """Per direction: one layout (used by both layers) + streams + inv."""
    src = edge[0].astype(np.int64)
    dst = edge[1].astype(np.int64)
    core = dst // NPC
    ldst = dst - core * NPC
    w = ldst // 128
    woff = ldst % 128

    cnt = np.bincount(core * NPCP + ldst, minlength=NCORE * NPCP).reshape(NCORE, NPCP)
    inv = np.where(cnt > 0, 1.0 / np.maximum(cnt, 1), 0.0).astype(np.float32)
    inv_t = np.ascontiguousarray(
        inv.reshape(NCORE, W, 128).transpose(0, 2, 1))  # [8, 128, W]

    # src indices into the AllGathered table [(src//NPC)*NPCP + src%NPC]
    pos = (src // NPC) * NPCP + (src % NPC)
    q = pos // QSH
    iq = pos - q * QSH
    counts = np.bincount(((core * W + w) * 4 + q).astype(np.int64),
                         minlength=NCORE * W * 4).reshape(NCORE, W, 4)
    layout = PassLayout(counts.max(axis=0), W)
    idx_wrap, dst_t = _build_pass(q, iq, core, w, woff, layout)
    return layout, idx_wrap, dst_t, inv_t


# -------------------------------------------------------------- bass builder

def _build_nc(NPC, NPCP, W, QSH, NTH, NBMAX, layouts):
    """layouts: dict[d] -> PassLayout"""
    f32 = mybir.dt.float32
    bf16 = mybir.dt.bfloat16
    AF = mybir.ActivationFunctionType
    ALU = mybir.AluOpType

    nc = bacc.Bacc(None, target_bir_lowering=False, debug=False,
                   num_swdge_queues=2)
    t_x0T = nc.dram_tensor("x0T", [128, 2 * NPCP], bf16, kind="ExternalInput")
    t_iota2 = nc.dram_tensor("iota2", [128, 128], bf16, kind="ExternalInput")
    t_idx, t_dst, t_inv = {}, {}, {}
    for d in range(2):
        lo = layouts[d]
        t_idx[d] = nc.dram_tensor(f"idx_{d}", [128, lo.tot_idx // 16],
                                  mybir.dt.int16, kind="ExternalInput")
        t_dst[d] = nc.dram_tensor(f"dst_{d}", [128, lo.tot_col], f32,
                                  kind="ExternalInput")
        t_inv[d] = nc.dram_tensor(f"inv_{d}", [128, W], f32,
                                  kind="ExternalInput")
    names = ["in1", "in2", "out1", "out2"]
    t_w = {}
    for nm in names:
        d_in = D0 if nm.endswith("1") else HID
        t_w[nm + "_Wl"] = nc.dram_tensor(nm + "_Wl", [d_in, HID], f32, kind="ExternalInput")
        t_w[nm + "_Wr"] = nc.dram_tensor(nm + "_Wr", [d_in, HID], f32, kind="ExternalInput")
        t_w[nm + "_bl"] = nc.dram_tensor(nm + "_bl", [HID], f32, kind="ExternalInput")
    t_lin = nc.dram_tensor("lin_W", [2 * HID], f32, kind="ExternalInput")
    t_linb = nc.dram_tensor("lin_b", [128], f32, kind="ExternalInput")
    t_y = nc.dram_tensor("y", [NPCP], f32, kind="ExternalOutput")

    with TileContext(nc) as tc:
        with (
            tc.tile_pool(name="const", bufs=1) as constp,
            tc.tile_pool(name="stage", bufs=2) as stagep,
            tc.tile_pool(name="sS", bufs=6) as sp,
            tc.tile_pool(name="small", bufs=3) as smallp,
            tc.tile_pool(name="mns", bufs=6) as mnsp,
            tc.tile_pool(name="big", bufs=1) as bigp,
            tc.tile_pool(name="ps", bufs=6, space="PSUM") as psp,
            tc.tile_pool(name="pst", bufs=2, space="PSUM") as pstp,
            tc.tile_pool(name="dram", bufs=1, space="DRAM") as dramp,
        ):
            nc.gpsimd.load_library(mlp)
            # constants
            ident = constp.tile([128, 128], f32, tag="ident")
            make_identity(nc, ident[:])
            ident_bf = constp.tile([128, 128], bf16, tag="identbf")
            nc.vector.tensor_copy(out=ident_bf[:], in_=ident[:])
            iota = constp.tile([128, 128], bf16, tag="iota2")
            nc.sync.dma_start(out=iota[:], in_=t_iota2[:])
            zeros512 = constp.tile([128, 512], bf16, tag="z512")
            nc.vector.memset(zeros512[:], 0.0)
            invt = {}
            for d in range(2):
                invt[d] = constp.tile([128, W], f32, tag=f"inv{d}", name=f"inv{d}")
                nc.sync.dma_start(out=invt[d][:], in_=t_inv[d][:])
            # weights: load f32 then convert to bf16 once
            wt = {}
            for nm in names:
                for side in ("Wl", "Wr"):
                    src_t = t_w[nm + "_" + side]
                    a32 = smallp.tile([128, 128], f32, tag="w32")
                    nc.sync.dma_start(out=a32[:], in_=src_t[0:128, :])
                    a = constp.tile([128, 128], bf16, tag=f"{nm}{side}a",
                                    name=f"{nm}{side}a")
                    nc.vector.tensor_copy(out=a[:], in_=a32[:])
                    wt[nm + side + "a"] = a
                    if nm.endswith("1"):
                        b32 = smallp.tile([128, 128], f32, tag="w32")
                        nc.sync.dma_start(out=b32[0:16, :], in_=src_t[128:144, :])
                        b = constp.tile([128, 128], bf16, tag=f"{nm}{side}b",
                                        name=f"{nm}{side}b")
                        nc.vector.tensor_copy(out=b[0:16, :], in_=b32[0:16, :])
                        wt[nm + side + "b"] = b
                bt = constp.tile([128, 1], f32, tag=f"{nm}bl", name=f"{nm}bl")
                nc.sync.dma_start(out=bt[:], in_=t_w[nm + "_bl"][:, None])
                wt[nm + "bl"] = bt
            lin_f = constp.tile([128, 2], f32, tag="linf")
            nc.sync.dma_start(out=lin_f[:], in_=t_lin.rearrange("(h p) -> p h", p=128))
            lin_bf = constp.tile([128, 2], bf16, tag="linbf")
            nc.vector.tensor_copy(out=lin_bf[:], in_=lin_f[:])
            linb_sb = constp.tile([128, 1], f32, tag="linb")
            nc.sync.dma_start(out=linb_sb[:], in_=t_linb[:, None])

            y_sb = constp.tile([128, W], f32, tag="ysb")

            h1T = [bigp.tile([128, NPCP], bf16, tag=f"h1T{d}", name=f"h1T{d}")
                   for d in range(2)]
            z_own = [dramp.tile([NPCP, HID], bf16, tag=f"zown{d}", name=f"zown{d}")
                     for d in range(2)]
            e_own = [dramp.tile([NPCP, HID], bf16, tag=f"eown{d}", name=f"eown{d}")
                     for d in range(2)]
            z_ag = [dramp.tile([NTH, HID], bf16, tag=f"zag{d}", name=f"zag{d}",
                               addr_space="Shared") for d in range(2)]
            e_ag = [dramp.tile([NTH, HID], bf16, tag=f"eag{d}", name=f"eag{d}",
                               addr_space="Shared") for d in range(2)]

            qctr = [0]
            NTILE = _ceil(NPCP, 512)

            def export_tile(srcT, src_off, nn, dram_own, node0):
                """srcT[:, src_off:+nn] (bf16, feature-major) -> node-major DRAM
                rows node0..node0+nn."""
                nwin = nn // 128
                hn = smallp.tile([128, 512], bf16, tag="hn")
                for k in range(nwin):
                    tp = pstp.tile([128, 256], bf16, tag="tr", name="tpbf")
                    nc.tensor.transpose(
                        tp[:, 0:128],
                        srcT[:, src_off + k * 128:src_off + (k + 1) * 128],
                        ident_bf[:])
                    nc.vector.tensor_copy(out=hn[:, k * 128:(k + 1) * 128],
                                          in_=tp[:, 0:128])
                w0 = node0 // 128
                nc.sync.dma_start(
                    out=dram_own[:].rearrange("(w p) j -> p w j", p=128)[:, w0:w0 + nwin, :],
                    in_=hn[:, 0:nn].rearrange("p (w j) -> p w j", j=128))

            def transform_pass(d, lhsT_list, src_rhs, dram_own):
                """out = sum_i lhsT_i^T @ rhs_i per 512-tile, export node-major."""
                for ti in range(NTILE):
                    n0 = ti * 512
                    nn = min(512, NPCP - n0)
                    ps = psp.tile([128, 512], f32, tag="red")
                    rhss = src_rhs(n0, nn)
                    for i, (lhsT, rhs) in enumerate(zip(lhsT_list, rhss)):
                        nc.tensor.matmul(ps[:, 0:nn], lhsT=lhsT, rhs=rhs,
                                         start=(i == 0),
                                         stop=(i == len(lhsT_list) - 1))
                    zt = smallp.tile([128, 512], bf16, tag="zt")
                    nc.scalar.activation(out=zt[:, 0:nn], in_=ps[:, 0:nn],
                                         func=AF.Copy)
                    export_tile(zt, 0, nn, dram_own, n0)

            def load_xa(n0, nn):
                xa = smallp.tile([128, 512], bf16, tag="xa")
                nc.sync.dma_start(out=xa[:, 0:nn], in_=t_x0T[:, n0:n0 + nn])
                xb = smallp.tile([128, 512], bf16, tag="xb")
                nc.sync.dma_start(out=xb[0:16, 0:nn],
                                  in_=t_x0T[0:16, NPCP + n0:NPCP + n0 + nn])
                return xa, xb

            def reduce_dense(d, table_ap, dense_mms, bias, out_cb):
                """Aggregation pass + fused dense layer.

                dense_mms(n0, nn) -> list of (lhsT, rhs) accumulated with the
                transposed means into one PSUM tile; out_cb(ps, n0, nn)
                consumes the finished PSUM (adding bias/relu)."""
                lo = layouts[d]
                for g in range(lo.NG):
                    wins = list(range(g * GW, min((g + 1) * GW, lo.W)))
                    nk = _ceil(len(wins), 4)
                    ptiles = [psp.tile([128, 512], f32, tag="red",
                                       name=f"agg{g}_{k}") for k in range(nk)]
                    segs = [s for s in lo.segs if s[0] == g]
                    # per-bank first/last matmul bookkeeping (PSUM groups are
                    # bank-granular: interleaved per-region groups lose
                    # contributions — probed on HW)
                    wcount = {wv: 0 for wv in wins}
                    kcount = [0] * nk
                    for (_, _, _, _, _, mms) in segs:
                        for (_, wv, _) in mms:
                            wcount[wv] += 1
                            kcount[wins.index(wv) // 4] += 1
                    kseen = [0] * nk
                    # zero each scatter bank with a PE matmul (opens the
                    # accumulation group covering the full tile; cheaper than
                    # DVE memset on the critical path)
                    for k in range(nk):
                        nc.tensor.matmul(ptiles[k][:, 0:512],
                                         lhsT=ident_bf[:], rhs=zeros512[:],
                                         start=True, stop=(kcount[k] == 0))
                    for (gg, q, ip, seglen, dc, mms) in segs:
                        nb = seglen // 128
                        ncols = len(mms)
                        idxs = smallp.tile([128, seglen // 16], mybir.dt.int16,
                                           tag="idxs")
                        nc.sync.dma_start(
                            out=idxs[:],
                            in_=t_idx[d][:, ip // 16: ip // 16 + seglen // 16])
                        dstc = smallp.tile([128, max(ncols, 1)], f32, tag="dstc")
                        if ncols:
                            nc.sync.dma_start(
                                out=dstc[:, 0:ncols],
                                in_=t_dst[d][:, dc:dc + ncols])
                        stage = stagep.tile([128, nb * 128], bf16, tag="stage")
                        nc.gpsimd.dma_gather(
                            stage[:].rearrange("p (b e) -> p b e", e=128),
                            table_ap[q * QSH:(q + 1) * QSH],
                            idxs[:],
                            seglen, seglen, 128,
                            single_packet=False,
                            queue_num=qctr[0] % 2,
                        )
                        qctr[0] += 1
                        for (b, wv, col) in mms:
                            wl = wins.index(wv)
                            k = wl // 4
                            kseen[k] += 1
                            S = sp.tile([128, 128], bf16, tag="S")
                            nc.vector.tensor_scalar(
                                out=S[:], in0=iota[:],
                                scalar1=dstc[:, col:col + 1], scalar2=None,
                                op0=ALU.is_equal)
                            nc.tensor.matmul(
                                ptiles[k][:, (wl % 4) * 128:(wl % 4) * 128 + 128],
                                lhsT=S[:],
                                rhs=stage[:, b * 128:b * 128 + 128],
                                start=False,
                                stop=(kseen[k] == kcount[k]))
                    # evictions + fused dense per 4-window node tile
                    for k in range(nk):
                        wk = wins[4 * k:4 * k + 4]
                        n0 = wk[0] * 128
                        nn = len(wk) * 128
                        mts = []
                        for j, wv in enumerate(wk):
                            wl = 4 * k + j
                            mn = mnsp.tile([128, 128], f32, tag="mn")
                            nc.scalar.activation(
                                out=mn[:],
                                in_=ptiles[k][:, (wl % 4) * 128:(wl % 4) * 128 + 128],
                                func=AF.Copy,
                                scale=invt[d][:, wv:wv + 1])
                            mts.append(mn)
                        ps = psp.tile([128, 512], f32, tag="red",
                                      name=f"dense{g}_{k}")
                        dm = dense_mms(n0, nn)
                        for i, (lhsT, rhs) in enumerate(dm):
                            nc.tensor.matmul(ps[:, 0:nn], lhsT=lhsT, rhs=rhs,
                                             start=(i == 0), stop=False)
                        for j, mn in enumerate(mts):
                            nc.tensor.matmul(
                                ps[:, j * 128:(j + 1) * 128],
                                lhsT=mn[:], rhs=ident[:],
                                is_transpose=True,
                                start=False, stop=(j == len(mts) - 1))
                        out_cb(ps, n0, nn)

            for d in range(2):
                nm1, nm2 = names[2 * d], names[2 * d + 1]
                # ---- z-pass: z = x0 @ Wl1, export + AllGather ----
                transform_pass(
                    d,
                    [wt[nm1 + "Wla"][:], wt[nm1 + "Wlb"][0:16, :]],
                    lambda n0, nn: [x[0:128, 0:nn] if i == 0 else x[0:16, 0:nn]
                                    for i, x in enumerate(load_xa(n0, nn))],
                    z_own[d])
                nc.gpsimd.collective_compute(
                    "AllGather", mybir.AluOpType.bypass,
                    replica_groups=[list(range(NCORE))],
                    ins=[z_own[d][:]], outs=[z_ag[d][:]])

            for d in range(2):
                nm1, nm2 = names[2 * d], names[2 * d + 1]

                # ---- layer 1: mean(z) + x0 @ Wr1, relu -> h1T ----
                def l1_dense(n0, nn):
                    xa, xb = load_xa(n0, nn)
                    return [(wt[nm1 + "Wra"][:], xa[:, 0:nn]),
                            (wt[nm1 + "Wrb"][0:16, :], xb[0:16, 0:nn])]

                def l1_out(ps, n0, nn, d=d, nm1=nm1):
                    nc.scalar.activation(out=h1T[d][:, n0:n0 + nn],
                                         in_=ps[:, 0:nn],
                                         func=AF.Relu,
                                         bias=wt[nm1 + "bl"][:, 0:1], scale=1.0)

                reduce_dense(d, z_ag[d][:], l1_dense, wt[nm1 + "bl"], l1_out)

                # ---- e1-pass: e1 = h1 @ Wl2, export + AllGather ----
                transform_pass(
                    d,
                    [wt[nm2 + "Wla"][:]],
                    lambda n0, nn: [h1T[d][:, n0:n0 + nn]],
                    e_own[d])
                nc.gpsimd.collective_compute(
                    "AllGather", mybir.AluOpType.bypass,
                    replica_groups=[list(range(NCORE))],
                    ins=[e_own[d][:]], outs=[e_ag[d][:]])

            for d in range(2):
                nm1, nm2 = names[2 * d], names[2 * d + 1]

                # ---- layer 2: mean(e1) + h1 @ Wr2, relu, residual ----
                def l2_dense(n0, nn, d=d, nm2=nm2):
                    return [(wt[nm2 + "Wra"][:], h1T[d][:, n0:n0 + nn])]

                def l2_out(ps, n0, nn, d=d, nm2=nm2):
                    h2t = smallp.tile([128, 512], bf16, tag="h2t")
                    nc.scalar.activation(out=h2t[:, 0:nn], in_=ps[:, 0:nn],
                                         func=AF.Relu,
                                         bias=wt[nm2 + "bl"][:, 0:1], scale=1.0)
                    nc.vector.tensor_add(out=h1T[d][:, n0:n0 + nn],
                                         in0=h1T[d][:, n0:n0 + nn],
                                         in1=h2t[:, 0:nn])

                reduce_dense(d, e_ag[d][:], l2_dense, wt[nm2 + "bl"], l2_out)

            # ---------------- y = h_in @ lin[:128] + h_out @ lin[128:] + b ----
            for wv in range(W):
                yp = psp.tile([128, 512], f32, tag="red", name="yp")
                nc.tensor.matmul(yp[:, 0:1],
                                 lhsT=h1T[0][:, wv * 128:(wv + 1) * 128],
                                 rhs=lin_bf[:, 0:1], start=True, stop=False)
                nc.tensor.matmul(yp[:, 0:1],
                                 lhsT=h1T[1][:, wv * 128:(wv + 1) * 128],
                                 rhs=lin_bf[:, 1:2], start=False, stop=True)
                nc.scalar.activation(out=y_sb[:, wv:wv + 1], in_=yp[:, 0:1],
                                     func=AF.Copy)
            nc.vector.tensor_scalar(
                out=y_sb[:], in0=y_sb[:],
                scalar1=linb_sb[:, 0:1], scalar2=None,
                op0=ALU.add)
            nc.sync.dma_start(out=t_y.rearrange("(w p) -> p w", p=128), in_=y_sb[:])

    nc.compile()
    return nc


# ------------------------------------------------------------------ wrapper

def _prep_all(x, edge_in, edge_out, emb):
    N = x.shape[0]
    NPC = N // NCORE
    W = _ceil(NPC, 128)
    NPCP = W * 128
    QSH = 2 * NPCP
    NTH = NCORE * NPCP

    x = np.asarray(x, np.float32)
    emb = np.asarray(emb, np.float32)

    pre = {}
    for d, edge in enumerate((edge_in, edge_out)):
        pre[d] = _preprocess(np.asarray(edge), N, NPC, NPCP, W, QSH)

    NBMAX = max(max(pre[d][0].max_ncols for d in range(2)), 2)
    iota2 = np.broadcast_to(np.arange(128, dtype=np.float32),
                            (128, 128)).astype(BF)

    x0T = np.zeros((NCORE, 128, 2 * NPCP), np.float32)
    for c in range(NCORE):
        blk = np.zeros((NPCP, D0), np.float32)
        blk[:NPC, 0:128] = x[c * NPC:(c + 1) * NPC]
        blk[:NPC, 128:144] = emb[c * NPC:(c + 1) * NPC]
        x0T[c, :, :NPCP] = blk[:, 0:128].T
        x0T[c, 0:16, NPCP:] = blk[:, 128:144].T
    x0T = x0T.astype(BF)

    layouts = {d: pre[d][0] for d in range(2)}
    dims = dict(N=N, NPC=NPC, NPCP=NPCP, W=W, QSH=QSH, NTH=NTH, NBMAX=NBMAX)
    return dims, layouts, pre, x0T, iota2


def _in_maps(dims, pre, x0T, iota2, kw):
    maps = []
    for c in range(NCORE):
        m = {"x0T": np.ascontiguousarray(x0T[c]), "iota2": np.asarray(iota2)}
        for d in range(2):
            _, idx_wrap, dst_t, inv_t = pre[d]
            m[f"idx_{d}"] = np.ascontiguousarray(idx_wrap[c])
            m[f"dst_{d}"] = np.ascontiguousarray(dst_t[c])
            m[f"inv_{d}"] = np.ascontiguousarray(inv_t[c])
        for nm in ("in1", "in2", "out1", "out2"):
            m[nm + "_Wl"] = np.asarray(kw[nm + "_Wl"], np.float32)
            m[nm + "_Wr"] = np.asarray(kw[nm + "_Wr"], np.float32)
            m[nm + "_bl"] = np.asarray(kw[nm + "_bl"], np.float32)
        m["lin_W"] = np.asarray(kw["lin_W"], np.float32).reshape(-1)
        m["lin_b"] = np.full(128, np.asarray(kw["lin_b"], np.float32).reshape(-1)[0], np.float32)
        maps.append(m)
    return maps


def kernel(x, edge_in, edge_out, emb, **kw):
    from concourse.bass_utils import run_bass_kernel_spmd
    dims, layouts, pre, x0T, iota2 = _prep_all(x, edge_in, edge_out, emb)
    nc = _build_nc(dims["NPC"], dims["NPCP"], dims["W"], dims["QSH"],
                   dims["NTH"], dims["NBMAX"], layouts)
    maps = _in_maps(dims, pre, x0T, iota2, kw)
    res = run_bass_kernel_spmd(nc, maps, core_ids=list(range(NCORE)))
    NPC = dims["NPC"]
    y = np.empty(dims["N"], np.float32)
    for c in range(NCORE):
        y[c * NPC:(c + 1) * NPC] = res.results[c]["y"][:NPC]
    return y
